# revision 1
# baseline (speedup 1.0000x reference)
"""CnnReservoirPolicy Trainium2 kernel (8-core SPMD).

Sharding: batch element b -> core b (32 CNN frames/core). BatchNorm batch
statistics (training mode, over the full 256-frame batch) are the only
cross-core dependency: 4 tiny AllReduces. Each core then runs the
reservoir scan + readout head for its own batch element; the host
assembles the 8 per-core output rows.

conv1 (7x7 s2) runs as a GEMM with M = 64ch x 2 output-row-parity and
K = 189 = (3c x 9yrel x 7kx); its im2col tensor is built host-side.
conv2 packs kx-pairs into K=128 via an x-shifted duplicate of its input.
All matmuls bf16 with fp32 PSUM accumulation. maxpool runs on raw conv1
outputs (commutes with the positive-scale BN affine + relu).
"""
import os
import sys

for _p in ("/opt/trn_rl_repo", os.path.expanduser("~/.axon_site/_ro/trn_rl_repo")):
    if os.path.isdir(_p) and _p not in sys.path:
        sys.path.insert(0, _p)

import numpy as np
import ml_dtypes

import concourse.bass as bass
import concourse.mybir as mybir
import concourse.tile as tile
from concourse import bacc
from concourse.bass_utils import run_bass_kernel_spmd

BF16 = ml_dtypes.bfloat16
F32 = mybir.dt.float32
BF = mybir.dt.bfloat16
AX = mybir.AxisListType
ALU = mybir.AluOpType
AF = mybir.ActivationFunctionType

ALPHA = 0.5
BN_EPS = 1e-5
NI, T = 2, 16          # num_images, n_obs_steps
FPC = NI * T           # frames per core = 32
K1 = 189               # conv1 GEMM contraction (3c x 9yrel x 7kx)
K1A = 128
K1B = K1 - K1A         # 61


# ----------------------------------------------------------------- host packing

def _bf(x):
    return np.ascontiguousarray(np.asarray(x, np.float32)).astype(BF16)


def conv1_im2col(imgs):
    """imgs (F,3,112,112) f32 -> (189, F, 28, 56) bf16.

    T[(c,yr,kx), f, r, x] = pad3(imgs)[f, c, 4r+yr, 2x+kx]
    (output row oy = 2r+ry pairs with weight column block ry.)
    """
    F = imgs.shape[0]
    p = np.pad(np.asarray(imgs, np.float32), ((0, 0), (0, 0), (3, 3), (3, 3)))
    pb = p.astype(BF16)
    Tm = np.empty((3, 9, 7, F, 28, 56), dtype=BF16)
    for yr in range(9):
        for kx in range(7):
            Tm[:, yr, kx] = pb[:, :, yr:yr + 112:4, kx:kx + 112:2].transpose(1, 0, 2, 3)
    return Tm.reshape(K1, F, 28, 56)


def pack_w1(w1):
    """w1 (64,3,7,7) -> (189, 128); col = ry*64+co; k=(c,yr,kx), ky=yr-2ry."""
    wp = np.zeros((3, 9, 7, 128), dtype=np.float32)
    for ry in range(2):
        for ky in range(7):
            yr = 2 * ry + ky
            wp[:, yr, :, ry * 64:(ry + 1) * 64] = \
                np.asarray(w1, np.float32)[:, :, ky, :].transpose(1, 2, 0)
    return _bf(wp.reshape(K1, 128))


def pack_w2(w2):
    """w2 (128,64,3,3) -> A [128=(d,ci),3ky,128co], B [64ci,3ky,128co]."""
    w2 = np.asarray(w2, np.float32)
    wA = np.zeros((128, 3, 128), dtype=np.float32)
    wB = np.zeros((64, 3, 128), dtype=np.float32)
    for ky in range(3):
        for d in range(2):
            wA[d * 64:(d + 1) * 64, ky, :] = w2[:, :, ky, d].T
        wB[:, ky, :] = w2[:, :, ky, 2].T
    return _bf(wA), _bf(wB)


def pack_w3(w3):
    """w3 (256,128,3,3) -> [128ci, 9tap, 2m, 128co]."""
    w3 = np.asarray(w3, np.float32)
    wp = np.empty((128, 9, 2, 128), dtype=np.float32)
    for tap in range(9):
        ky, kx = tap // 3, tap % 3
        for m in range(2):
            wp[:, tap, m, :] = w3[m * 128:(m + 1) * 128, :, ky, kx].T
    return _bf(wp)


def pack_w4(w4):
    """w4 (512,256,3,3) -> [128ci, 9tap, 2h, 4m, 128co]."""
    w4 = np.asarray(w4, np.float32)
    wp = np.empty((128, 9, 2, 4, 128), dtype=np.float32)
    for tap in range(9):
        ky, kx = tap // 3, tap % 3
        for h in range(2):
            for m in range(4):
                wp[:, tap, h, m, :] = \
                    w4[m * 128:(m + 1) * 128, h * 128:(h + 1) * 128, ky, kx].T
    return _bf(wp)


def pack_win(Win):
    """Win (1024,576) -> img [128, 4ki, 8m, 128], state [64, 8m, 128]."""
    Win = np.asarray(Win, np.float32)
    wi = np.empty((128, 4, 8, 128), dtype=np.float32)
    ws = np.empty((64, 8, 128), dtype=np.float32)
    for m in range(8):
        for ki in range(4):
            wi[:, ki, m, :] = Win[m * 128:(m + 1) * 128, ki * 128:(ki + 1) * 128].T
        ws[:, m, :] = Win[m * 128:(m + 1) * 128, 512:].T
    return _bf(wi), _bf(ws)


def pack_wres(Wres):
    """Wres (1024,1024) -> [128, 8k, 8m, 128]: [p,k,m,q] = Wres[m*128+q, k*128+p]."""
    Wres = np.asarray(Wres, np.float32)
    wp = np.empty((128, 8, 8, 128), dtype=np.float32)
    for k in range(8):
        for m in range(8):
            wp[:, k, m, :] = Wres[m * 128:(m + 1) * 128, k * 128:(k + 1) * 128].T
    return _bf(wp)


def pack_hw1(hW1):
    """hW1 (512,16384) -> [16t, 8m, 128p, 4mo, 128q]:
    [t,m,p,mo,q] = hW1[mo*128+q, t*1024 + m*128 + p]."""
    hW1 = np.asarray(hW1, np.float32)
    wp = np.empty((T, 8, 128, 4, 128), dtype=np.float32)
    for t in range(T):
        for m in range(8):
            for mo in range(4):
                wp[t, m, :, mo, :] = hW1[mo * 128:(mo + 1) * 128,
                                         t * 1024 + m * 128: t * 1024 + (m + 1) * 128].T
    return _bf(wp)


def pack_hw2(hW2):
    """hW2 (112,512) -> [128, 4mo, 112]: [p,mo,j] = hW2[j, mo*128+p]."""
    hW2 = np.asarray(hW2, np.float32)
    wp = np.empty((128, 4, 112), dtype=np.float32)
    for mo in range(4):
        wp[:, mo, :] = hW2[:, mo * 128:(mo + 1) * 128].T
    return _bf(wp)


# ----------------------------------------------------------------- program

def build_program(n_cores):
    nc = bacc.Bacc("TRN2", target_bir_lowering=False, debug=False,
                   num_devices=n_cores)
    core_ids = list(range(n_cores))
    use_cc = n_cores > 1

    ein = lambda name, shape, dt=BF: nc.dram_tensor(name, shape, dt, kind="ExternalInput")
    t1_d = ein("t1", [K1, FPC, 28, 56])
    w1_d = ein("w1p", [K1, 128])
    w2A_d = ein("w2A", [128, 3, 128])
    w2B_d = ein("w2B", [64, 3, 128])
    w3_d = ein("w3p", [128, 9, 2, 128])
    w4_d = ein("w4p", [128, 9, 2, 4, 128])
    wi_d = ein("wip", [128, 4, 8, 128])
    ws_d = ein("wsp", [64, 8, 128])
    wr_d = ein("wrp", [128, 8, 8, 128])
    h1_d = ein("h1p", [T, 8, 128, 4, 128])
    h2_d = ein("h2p", [128, 4, 112])
    st_d = ein("stT", [14, T])
    swt_d = ein("swT", [14, 64])
    gb1_d = ein("gb1", [64, 2], F32)
    gb2_d = ein("gb2", [128, 2], F32)
    gb3_d = ein("gb3", [128, 4], F32)    # cols: g_h0, g_h1, b_h0, b_h1
    gb4_d = ein("gb4", [128, 8], F32)    # cols: g x4m then b x4m
    sb_d = ein("sbv", [64, 1], F32)
    bres_d = ein("bres", [128, 8], F32)
    hb1_d = ein("hb1p", [128, 4], F32)
    hb2_d = ein("hb2p", [112, 1], F32)

    out_d = nc.dram_tensor("out", [112, 1], F32, kind="ExternalOutput")

    if use_cc:
        cc_i = [nc.dram_tensor(f"cc{i}i", s, F32) for i, s in
                enumerate(([64, 2], [128, 2], [128, 4], [128, 8]))]
        cc_o = [nc.dram_tensor(f"cc{i}o", s, F32, addr_space="Shared") for i, s in
                enumerate(([64, 2], [128, 2], [128, 4], [128, 8]))]

    BTOT = n_cores
    CNT = [float(BTOT * NI * T * hw) for hw in (56 * 56, 14 * 14, 7 * 7, 4 * 4)]

    with tile.TileContext(nc) as tc:
        with (
            tc.tile_pool(name="wts", bufs=1) as wts,
            tc.tile_pool(name="acts", bufs=1) as acts,
            tc.tile_pool(name="tchunk", bufs=2) as tchunk,
            tc.tile_pool(name="hstream", bufs=4) as hstream,
            tc.tile_pool(name="ps", bufs=7, space="PSUM") as ps,
            tc.tile_pool(name="ev", bufs=3) as ev,
            tc.tile_pool(name="misc", bufs=1) as misc,
        ):
            def load(dram, shape, dtype=BF):
                tl = wts.tile(shape, dtype, tag=dram.name + "_sb")
                nc.sync.dma_start(tl[:], dram[:])
                return tl

            w1a = wts.tile([K1A, 128], BF, tag="w1a")
            nc.sync.dma_start(w1a[:], t1_slice := w1_d[0:K1A, :])
            w1b = wts.tile([K1B, 128], BF, tag="w1b")
            nc.sync.dma_start(w1b[:], w1_d[K1A:K1, :])
            w2A = load(w2A_d, [128, 3, 128])
            w2B = load(w2B_d, [64, 3, 128])
            w3s = load(w3_d, [128, 9, 2, 128])
            w4s = load(w4_d, [128, 9, 2, 4, 128])
            wis = load(wi_d, [128, 4, 8, 128])
            wss = load(ws_d, [64, 8, 128])
            wrs = load(wr_d, [128, 8, 8, 128])
            h2s = load(h2_d, [128, 4, 112])
            sts = load(st_d, [14, T])
            swts = load(swt_d, [14, 64])
            gb1 = load(gb1_d, [64, 2], F32)
            gb2 = load(gb2_d, [128, 2], F32)
            gb3 = load(gb3_d, [128, 4], F32)
            gb4 = load(gb4_d, [128, 8], F32)
            sbv = load(sb_d, [64, 1], F32)
            bres = load(bres_d, [128, 8], F32)
            hb1 = load(hb1_d, [128, 4], F32)
            hb2 = load(hb2_d, [112, 1], F32)

            A2 = acts.tile([128, FPC, 30, 30], BF)
            A3 = acts.tile([128, FPC, 16, 16], BF)
            A4 = acts.tile([128, 2, FPC, 9, 9], BF)
            Y4 = acts.tile([128, 4, FPC, 16], BF)
            nc.gpsimd.memset(A2[:], 0.0)
            nc.gpsimd.memset(A3[:], 0.0)
            nc.gpsimd.memset(A4[:], 0.0)

            s1acc = acts.tile([128, FPC * 4], F32)
            q1acc = acts.tile([128, FPC], F32)
            s2acc = acts.tile([128, 32], F32)
            q2acc = acts.tile([128, 32], F32)
            s3acc = acts.tile([128, 64], F32)
            q3acc = acts.tile([128, 64], F32)
            s4acc = acts.tile([128, 4], F32)
            q4acc = acts.tile([128, 4], F32)

            # ================ conv1 + maxpool, per frame
            r_groups = [(0, 8), (8, 16), (16, 24), (24, 28)]
            for f in range(FPC):
                ta = tchunk.tile([K1A, 28, 56], BF, tag="ta")
                nc.sync.dma_start(ta[:], t1_d[0:K1A, f, :, :])
                tb = tchunk.tile([K1B, 28, 56], BF, tag="tb")
                nc.sync.dma_start(tb[:], t1_d[K1A:K1, f, :, :])
                yf = ev.tile([128, 28, 56], BF, tag="yf")
                for gi, (r0, r1) in enumerate(r_groups):
                    pt = ps.tile([128, r1 - r0, 56], F32, tag="ps")
                    nc.tensor.matmul(pt[:], w1a[:], ta[:, r0:r1, :],
                                     start=True, stop=False)
                    nc.tensor.matmul(pt[:], w1b[:], tb[:, r0:r1, :],
                                     start=False, stop=True)
                    nc.scalar.activation(yf[:, r0:r1, :], pt[:], AF.Copy,
                                         accum_out=s1acc[:, f * 4 + gi: f * 4 + gi + 1])
                sq = ev.tile([128, 28, 56], BF, tag="sq")
                nc.scalar.activation(sq[:], yf[:], AF.Square,
                                     accum_out=q1acc[:, f:f + 1])
                # maxpool 3x3 s2 p1; rows y=2r+ry live as (partition ry, free r)
                ax = ev.tile([128, 28, 28], BF, tag="ax")
                nc.vector.tensor_tensor(out=ax[:], in0=yf[:, :, 0:56:2],
                                        in1=yf[:, :, 1:56:2], op=ALU.max)
                px = ev.tile([128, 28, 28], BF, tag="px")
                nc.vector.tensor_copy(px[:, :, 0:1], ax[:, :, 0:1])
                nc.vector.tensor_tensor(out=px[:, :, 1:28], in0=ax[:, :, 1:28],
                                        in1=yf[:, :, 1:54:2], op=ALU.max)
                # copy upper-half rows to base-0 (walrus: TT inputs must
                # share base partition)
                pxb = ev.tile([64, 28, 28], BF, tag="pxb")
                nc.vector.tensor_copy(pxb[:], px[64:128, :, :])
                cy = ev.tile([64, 28, 28], BF, tag="cy")
                nc.vector.tensor_tensor(out=cy[:], in0=px[0:64, :, :],
                                        in1=pxb[:], op=ALU.max)
                nc.vector.tensor_copy(A2[0:64, f, 1:2, 1:29], cy[:, 0:1, :])
                nc.vector.tensor_tensor(out=A2[0:64, f, 2:29, 1:29],
                                        in0=cy[:, 1:28, :],
                                        in1=pxb[:, 0:27, :], op=ALU.max)

            # ---------------- BN helpers
            def bn_finalize(s_red, q_red, gb, cnt, P, K, lname):
                m = misc.tile([P, K], F32, tag=lname + "_m")
                nc.vector.tensor_scalar_mul(m[:], s_red[:], 1.0 / cnt)
                msq = misc.tile([P, K], F32, tag=lname + "_msq")
                nc.vector.tensor_mul(msq[:], m[:], m[:])
                v = misc.tile([P, K], F32, tag=lname + "_v")
                nc.vector.scalar_tensor_tensor(out=v[:], in0=q_red[:], scalar=1.0 / cnt,
                                               in1=msq[:], op0=ALU.mult,
                                               op1=ALU.subtract)
                ve = misc.tile([P, K], F32, tag=lname + "_ve")
                nc.vector.tensor_scalar_add(ve[:], v[:], float(BN_EPS))
                rc = misc.tile([P, K], F32, tag=lname + "_rc")
                nc.vector.reciprocal(rc[:], ve[:])
                rinv = misc.tile([P, K], F32, tag=lname + "_rinv")
                nc.scalar.activation(rinv[:], rc[:], AF.Sqrt)
                a = misc.tile([P, K], F32, tag=lname + "_a")
                nc.vector.tensor_mul(a[:], rinv[:], gb[:, 0:K])
                tmv = misc.tile([P, K], F32, tag=lname + "_tmv")
                nc.vector.tensor_mul(tmv[:], m[:], a[:])
                b = misc.tile([P, K], F32, tag=lname + "_b")
                nc.vector.tensor_sub(b[:], gb[:, K:2 * K], tmv[:])
                return a, b

            def allreduce(idx, src, P, K, lname):
                if not use_cc:
                    return src
                nc.sync.dma_start(cc_i[idx][:], src[:])
                nc.gpsimd.collective_compute(
                    "AllReduce", ALU.add, ins=[cc_i[idx][:]], outs=[cc_o[idx][:]],
                    replica_groups=[core_ids])
                g = misc.tile([P, K], F32, tag=lname + "_g")
                nc.sync.dma_start(g[:], cc_o[idx][:])
                return g

            # ---------------- BN1 -> A2
            s1r = misc.tile([128, 1], F32, tag="s1r")
            nc.vector.reduce_sum(s1r[:], s1acc[:], axis=AX.X)
            q1r = misc.tile([128, 1], F32, tag="q1r")
            nc.vector.reduce_sum(q1r[:], q1acc[:], axis=AX.X)
            st1 = misc.tile([64, 2], F32, tag="st1")
            s1hi = misc.tile([64, 2], F32, tag="s1hi")
            nc.vector.tensor_copy(s1hi[:, 0:1], s1r[64:128, :])
            nc.vector.tensor_copy(s1hi[:, 1:2], q1r[64:128, :])
            nc.vector.tensor_add(st1[:, 0:1], s1r[0:64, :], s1hi[:, 0:1])
            nc.vector.tensor_add(st1[:, 1:2], q1r[0:64, :], s1hi[:, 1:2])
            g1 = allreduce(0, st1, 64, 2, "l1")
            a1, b1 = bn_finalize(g1[:, 0:1], g1[:, 1:2], gb1, CNT[0], 64, 1, "l1")
            for f in range(FPC):
                nc.scalar.activation(A2[0:64, f, 1:29, 1:29], A2[0:64, f, 1:29, 1:29],
                                     AF.Relu, bias=b1[:, 0:1], scale=a1[:, 0:1])
            for f in range(FPC):
                nc.vector.tensor_copy(A2[64:128, f, :, 0:29], A2[0:64, f, :, 1:30])

            # ================ conv2
            for g in range(FPC // 2):
                f0 = g * 2
                pt = ps.tile([128, 2, 14, 14], F32, tag="ps")
                for ky in range(3):
                    nc.tensor.matmul(pt[:], w2A[:, ky, :],
                                     A2[:, f0:f0 + 2, ky:ky + 28:2, 0:28:2],
                                     start=(ky == 0), stop=False)
                    nc.tensor.matmul(pt[:], w2B[:, ky, :],
                                     A2[0:64, f0:f0 + 2, ky:ky + 28:2, 2:30:2],
                                     start=False, stop=(ky == 2))
                for j in range(2):
                    nc.scalar.activation(A3[:, f0 + j, 1:15, 1:15], pt[:, j, :, :],
                                         AF.Copy,
                                         accum_out=s2acc[:, 2 * g + j:2 * g + j + 1])
                    sq2 = ev.tile([128, 14, 14], BF, tag="sqs")
                    nc.scalar.activation(sq2[:], A3[:, f0 + j, 1:15, 1:15], AF.Square,
                                         accum_out=q2acc[:, 2 * g + j:2 * g + j + 1])

            st2 = misc.tile([128, 2], F32, tag="st2")
            nc.vector.reduce_sum(st2[:, 0:1], s2acc[:], axis=AX.X)
            nc.vector.reduce_sum(st2[:, 1:2], q2acc[:], axis=AX.X)
            g2 = allreduce(1, st2, 128, 2, "l2")
            a2, b2 = bn_finalize(g2[:, 0:1], g2[:, 1:2], gb2, CNT[1], 128, 1, "l2")
            for f in range(FPC):
                nc.scalar.activation(A3[:, f, 1:15, 1:15], A3[:, f, 1:15, 1:15],
                                     AF.Relu, bias=b2[:, 0:1], scale=a2[:, 0:1])

            # ================ conv3
            for m in range(2):
                for g in range(4):
                    f0 = g * 8
                    pt = ps.tile([128, 8, 7, 7], F32, tag="ps")
                    for tap in range(9):
                        ky, kx = tap // 3, tap % 3
                        nc.tensor.matmul(pt[:], w3s[:, tap, m, :],
                                         A3[:, f0:f0 + 8, ky:ky + 14:2, kx:kx + 14:2],
                                         start=(tap == 0), stop=(tap == 8))
                    for j in range(8):
                        ci = m * FPC + f0 + j
                        nc.scalar.activation(
                            A4[:, m, f0 + j, 1:8, 1:8], pt[:, j, :, :], AF.Copy,
                            accum_out=s3acc[:, ci:ci + 1])
                        sq3 = ev.tile([128, 7, 7], BF, tag="sqs")
                        nc.scalar.activation(sq3[:], A4[:, m, f0 + j, 1:8, 1:8],
                                             AF.Square, accum_out=q3acc[:, ci:ci + 1])

            st3 = misc.tile([128, 4], F32, tag="st3")
            for m in range(2):
                nc.vector.reduce_sum(st3[:, m:m + 1], s3acc[:, m * FPC:(m + 1) * FPC],
                                     axis=AX.X)
                nc.vector.reduce_sum(st3[:, 2 + m:3 + m], q3acc[:, m * FPC:(m + 1) * FPC],
                                     axis=AX.X)
            g3 = allreduce(2, st3, 128, 4, "l3")
            a3, b3 = bn_finalize(g3[:, 0:2], g3[:, 2:4], gb3, CNT[2], 128, 2, "l3")
            for h in range(2):
                for f in range(FPC):
                    nc.scalar.activation(
                        A4[:, h, f, 1:8, 1:8], A4[:, h, f, 1:8, 1:8],
                        AF.Relu, bias=b3[:, h:h + 1], scale=a3[:, h:h + 1])

            # ================ conv4
            for m in range(4):
                pt = ps.tile([128, FPC, 4, 4], F32, tag="ps")
                first = True
                for tap in range(9):
                    ky, kx = tap // 3, tap % 3
                    for h in range(2):
                        nc.tensor.matmul(pt[:], w4s[:, tap, h, m, :],
                                         A4[:, h, :, ky:ky + 7:2, kx:kx + 7:2],
                                         start=first, stop=(tap == 8 and h == 1))
                        first = False
                nc.scalar.activation(
                    Y4[:, m, :, :].rearrange("p f (y x) -> p f y x", y=4),
                    pt[:], AF.Copy, accum_out=s4acc[:, m:m + 1])
                sq4 = ev.tile([128, FPC, 16], BF, tag="sqs")
                nc.scalar.activation(sq4[:], Y4[:, m, :, :], AF.Square,
                                     accum_out=q4acc[:, m:m + 1])

            st4 = misc.tile([128, 8], F32, tag="st4")
            nc.vector.tensor_copy(st4[:, 0:4], s4acc[:])
            nc.vector.tensor_copy(st4[:, 4:8], q4acc[:])
            g4 = allreduce(3, st4, 128, 8, "l4")
            a4, b4 = bn_finalize(g4[:, 0:4], g4[:, 4:8], gb4, CNT[3], 128, 4, "l4")

            # BN4+relu -> avgpool(16px) -> image-mean -> feats [128,(4m,16t)]
            S4 = acts.tile([128, 4, FPC], F32)
            for m in range(4):
                r4m = ev.tile([128, FPC, 16], F32, tag="r4m")
                nc.scalar.activation(r4m[:], Y4[:, m, :, :], AF.Relu,
                                     bias=b4[:, m:m + 1], scale=a4[:, m:m + 1])
                nc.vector.reduce_sum(S4[:, m, :], r4m[:], axis=AX.X)
            fsum = acts.tile([128, 4, T], F32)
            nc.vector.tensor_add(fsum[:], S4[:, :, 0:T], S4[:, :, T:FPC])
            FIb = acts.tile([128, 4, T], BF)
            nc.vector.tensor_scalar_mul(FIb[:], fsum[:], 1.0 / 32.0)

            # ================ state embedding (own b): SE [64, 16] bf16
            pse = ps.tile([64, T], F32, tag="ps")
            nc.tensor.matmul(pse[:], swts[:], sts[:], start=True, stop=True)
            SE = acts.tile([64, T], BF)
            nc.scalar.activation(SE[:], pse[:], AF.Relu, bias=sbv[:, 0:1])

            # ================ xin[p, t, m] = (Win @ step_inputs)_t + b_res
            xin = acts.tile([128, T, 8], F32)
            for m in range(8):
                px = ps.tile([128, T], F32, tag="ps")
                for ki in range(4):
                    nc.tensor.matmul(px[:], wis[:, ki, m, :], FIb[:, ki, :],
                                     start=(ki == 0), stop=False)
                nc.tensor.matmul(px[:], wss[:, m, :], SE[:], start=False, stop=True)
                nc.scalar.activation(xin[:, :, m], px[:], AF.Identity,
                                     bias=bres[:, m:m + 1])

            # ================ reservoir scan (own b)
            r_hist = acts.tile([128, T, 8], F32)
            rb = acts.tile([128, 8], BF)
            rzero = acts.tile([128, 8], F32)
            nc.vector.memset(rb[:], 0.0)
            nc.vector.memset(rzero[:], 0.0)
            for t in range(T):
                pr = ps.tile([128, 8], F32, tag="ps")
                for m in range(8):
                    for k in range(8):
                        nc.tensor.matmul(pr[:, m:m + 1], wrs[:, k, m, :],
                                         rb[:, k:k + 1], start=(k == 0), stop=(k == 7))
                u = ev.tile([128, 8], F32, tag="scan_u")
                nc.vector.tensor_add(u[:], pr[:], xin[:, t, :])
                th = ev.tile([128, 8], F32, tag="scan_th")
                nc.scalar.activation(th[:], u[:], AF.Tanh)
                v = ev.tile([128, 8], F32, tag="scan_v")
                r_prev = rzero[:] if t == 0 else r_hist[:, t - 1, :]
                nc.vector.tensor_add(v[:], th[:], r_prev)
                nc.vector.tensor_scalar_mul(r_hist[:, t, :], v[:], ALPHA)
                nc.vector.tensor_scalar_mul(rb[:], v[:], ALPHA)

            # ================ head (own b, full K)
            rh16 = acts.tile([128, T, 8], BF)
            nc.vector.tensor_copy(rh16[:], r_hist[:])
            Hr = acts.tile([128, 4], F32)
            phs = []
            for _mo in range(4):
                ph_t = ps.tile([128, 1], F32, tag="ps")
                phs.append(ph_t)
            for t in range(T):
                for m in range(8):
                    hc = hstream.tile([128, 4, 128], BF, tag="hc")
                    nc.sync.dma_start(hc[:], h1_d[t, m, :, :, :])
                    for mo in range(4):
                        nc.tensor.matmul(phs[mo][:], hc[:, mo, :],
                                         rh16[:, t, m:m + 1],
                                         start=(t == 0 and m == 0),
                                         stop=(t == T - 1 and m == 7))
            for mo in range(4):
                nc.scalar.activation(Hr[:, mo:mo + 1], phs[mo][:], AF.Relu,
                                     bias=hb1[:, mo:mo + 1])
            H16 = acts.tile([128, 4], BF)
            nc.vector.tensor_copy(H16[:], Hr[:])
            po = ps.tile([112, 1], F32, tag="ps")
            for mo in range(4):
                nc.tensor.matmul(po[:], h2s[:, mo, :], H16[:, mo:mo + 1],
                                 start=(mo == 0), stop=(mo == 3))
            OutS = acts.tile([112, 1], F32)
            nc.scalar.activation(OutS[:], po[:], AF.Identity, bias=hb2[:, 0:1])
            nc.sync.dma_start(out_d[:], OutS[:])

    nc.compile()
    return nc


# ----------------------------------------------------------------- host driver

_CACHE = {}


def make_in_map(inputs, core):
    b = core
    imgs = np.asarray(inputs["images_seq"], np.float32)[b].reshape(FPC, 3, 112, 112)
    wA, wB = pack_w2(inputs["w2"])
    wi, ws = pack_win(inputs["Win"])
    f32 = lambda x: np.asarray(x, np.float32)
    d = {
        "t1": conv1_im2col(imgs),
        "w1p": pack_w1(inputs["w1"]),
        "w2A": wA, "w2B": wB,
        "w3p": pack_w3(inputs["w3"]),
        "w4p": pack_w4(inputs["w4"]),
        "wip": wi, "wsp": ws,
        "wrp": pack_wres(inputs["Wres"]),
        "h1p": pack_hw1(inputs["hW1"]),
        "h2p": pack_hw2(inputs["hW2"]),
        "stT": _bf(f32(inputs["state_seq"])[b].T),
        "swT": _bf(f32(inputs["sW"]).T),
        "gb1": np.stack([f32(inputs["g1"]), f32(inputs["be1"])], axis=1),
        "gb2": np.stack([f32(inputs["g2"]), f32(inputs["be2"])], axis=1),
        "gb3": np.concatenate([f32(inputs["g3"]).reshape(2, 128).T,
                               f32(inputs["be3"]).reshape(2, 128).T], axis=1),
        "gb4": np.concatenate([f32(inputs["g4"]).reshape(4, 128).T,
                               f32(inputs["be4"]).reshape(4, 128).T], axis=1),
        "sbv": f32(inputs["sb"]).reshape(64, 1),
        "bres": f32(inputs["b_res"]).reshape(8, 128).T.copy(),
        "hb1p": f32(inputs["hb1"]).reshape(4, 128).T.copy(),
        "hb2p": f32(inputs["hb2"]).reshape(112, 1),
    }
    return d


def run(inputs, n_cores=8, **kw):
    core_ids = list(range(n_cores))
    if n_cores not in _CACHE:
        _CACHE[n_cores] = build_program(n_cores)
    nc = _CACHE[n_cores]
    in_maps = [make_in_map(inputs, c) for c in core_ids]
    res = run_bass_kernel_spmd(nc, in_maps, core_ids, **kw)
    rows = [np.asarray(res.results[c]["out"], np.float32).reshape(112)
            for c in core_ids]
    return np.stack(rows, axis=0), res


def kernel(**inputs):
    out, _ = run(inputs, n_cores=8)
    return out.reshape(8, 8, 14)



# revision 25
# speedup vs baseline: 1.1410x; 1.1410x over previous
"""CnnReservoirPolicy Trainium2 kernel (8-core SPMD).

Sharding: batch element b -> core b (32 CNN frames/core). BatchNorm batch
statistics (training mode, over the full 256-frame batch) are the only
cross-core CNN dependency: 4 tiny AllReduces. The readout head hW1 is
sharded by output rows (64 rows/core); each core computes its own batch
element's partial H, masked into a [64, 8] rank-column buffer and
AllReduced so every core gets the full 512-dim hidden vector.

conv1 (7x7 s2) runs as a GEMM with M = 64ch x 2 output-row-parity and
K = 189 = (3c x 9yrel x 7kx); its im2col tensor is built host-side.
conv2 packs kx-pairs into K=128 via an x-shifted duplicate of its input.
All matmuls bf16 with fp32 PSUM accumulation. maxpool runs on raw conv1
outputs (commutes with the positive-scale BN affine + relu).

The reservoir scan runs in s = 2r form (0.5*Wres folded host-side, and
0.5*hW1 for the readout) so the update is one scalar_tensor_tensor:
s_t = 0.5*s_{t-1} + tanh(xin_t + (0.5*Wres) s_{t-1}).
"""
import os
import sys

for _p in ("/opt/trn_rl_repo", os.path.expanduser("~/.axon_site/_ro/trn_rl_repo")):
    if os.path.isdir(_p) and _p not in sys.path:
        sys.path.insert(0, _p)

import numpy as np
import ml_dtypes

import concourse.bass as bass
import concourse.mybir as mybir
import concourse.tile as tile
from concourse import bacc
from concourse.bass_utils import run_bass_kernel_spmd

BF16 = ml_dtypes.bfloat16
F32 = mybir.dt.float32
BF = mybir.dt.bfloat16
AX = mybir.AxisListType
ALU = mybir.AluOpType
AF = mybir.ActivationFunctionType

ALPHA = 0.5
BN_EPS = 1e-5
DEBUG_DUMPS = False
NI, T = 2, 16          # num_images, n_obs_steps
FPC = NI * T           # frames per core = 32
K1 = 189               # conv1 GEMM contraction (3c x 9yrel x 7kx)
K1A = 128
K1B = K1 - K1A         # 61


# ----------------------------------------------------------------- host packing

def _bf(x):
    return np.ascontiguousarray(np.asarray(x, np.float32)).astype(BF16)


def conv1_im2col(imgs):
    """imgs (F,3,112,112) f32 -> (189, F, 28, 56) bf16.

    T[(c,yr,kx), f, r, x] = pad3(imgs)[f, c, 4r+yr, 2x+kx]
    (output row oy = 2r+ry pairs with weight column block ry.)
    """
    F = imgs.shape[0]
    p = np.pad(np.asarray(imgs, np.float32), ((0, 0), (0, 0), (3, 3), (3, 3)))
    pb = p.astype(BF16)
    Tm = np.empty((3, 9, 7, F, 28, 56), dtype=BF16)
    for yr in range(9):
        for kx in range(7):
            Tm[:, yr, kx] = pb[:, :, yr:yr + 112:4, kx:kx + 112:2].transpose(1, 0, 2, 3)
    return Tm.reshape(K1, F, 28, 56)


def pack_w1(w1):
    """w1 (64,3,7,7) -> (189, 128); col = ry*64+co; k=(c,yr,kx), ky=yr-2ry."""
    wp = np.zeros((3, 9, 7, 128), dtype=np.float32)
    for ry in range(2):
        for ky in range(7):
            yr = 2 * ry + ky
            wp[:, yr, :, ry * 64:(ry + 1) * 64] = \
                np.asarray(w1, np.float32)[:, :, ky, :].transpose(1, 2, 0)
    return _bf(wp.reshape(K1, 128))


def pack_w2(w2):
    """w2 (128,64,3,3) -> A [128=(d,ci),3ky,128co], B [64ci,3ky,128co]."""
    w2 = np.asarray(w2, np.float32)
    wA = np.zeros((128, 3, 128), dtype=np.float32)
    wB = np.zeros((64, 3, 128), dtype=np.float32)
    for ky in range(3):
        for d in range(2):
            wA[d * 64:(d + 1) * 64, ky, :] = w2[:, :, ky, d].T
        wB[:, ky, :] = w2[:, :, ky, 2].T
    return _bf(wA), _bf(wB)


def pack_w3(w3):
    """w3 (256,128,3,3) -> [128ci, 9tap, 2m, 128co]."""
    w3 = np.asarray(w3, np.float32)
    wp = np.empty((128, 9, 2, 128), dtype=np.float32)
    for tap in range(9):
        ky, kx = tap // 3, tap % 3
        for m in range(2):
            wp[:, tap, m, :] = w3[m * 128:(m + 1) * 128, :, ky, kx].T
    return _bf(wp)


def pack_w4(w4):
    """w4 (512,256,3,3) -> [128ci, 9tap, 2h, 4m, 128co]."""
    w4 = np.asarray(w4, np.float32)
    wp = np.empty((128, 9, 2, 4, 128), dtype=np.float32)
    for tap in range(9):
        ky, kx = tap // 3, tap % 3
        for h in range(2):
            for m in range(4):
                wp[:, tap, h, m, :] = \
                    w4[m * 128:(m + 1) * 128, h * 128:(h + 1) * 128, ky, kx].T
    return _bf(wp)


def pack_win(Win):
    """Win (1024,576) -> img [128, 4ki, 8m, 128], state [64, 8m, 128]."""
    Win = np.asarray(Win, np.float32)
    wi = np.empty((128, 4, 8, 128), dtype=np.float32)
    ws = np.empty((64, 8, 128), dtype=np.float32)
    for m in range(8):
        for ki in range(4):
            wi[:, ki, m, :] = Win[m * 128:(m + 1) * 128, ki * 128:(ki + 1) * 128].T
        ws[:, m, :] = Win[m * 128:(m + 1) * 128, 512:].T
    return _bf(wi), _bf(ws)


def pack_wres(Wres):
    """0.5*Wres (1024,1024) -> [128, 8k, 8m, 128]: [p,k,m,q] = W[m*128+q, k*128+p]."""
    Wres = 0.5 * np.asarray(Wres, np.float32)
    wp = np.empty((128, 8, 8, 128), dtype=np.float32)
    for k in range(8):
        for m in range(8):
            wp[:, k, m, :] = Wres[m * 128:(m + 1) * 128, k * 128:(k + 1) * 128].T
    return _bf(wp)


def pack_hw1_slice(hW1, core):
    """0.5*hW1 rows [64c:64c+64) -> [128p, 16t, 8m, 64j]:
    [p,t,m,j] = 0.5*hW1[64c+j, t*1024 + m*128 + p]."""
    hW1 = 0.5 * np.asarray(hW1, np.float32)[64 * core:64 * (core + 1), :]
    wp = np.empty((128, T, 8, 64), dtype=np.float32)
    for t in range(T):
        for m in range(8):
            wp[:, t, m, :] = hW1[:, t * 1024 + m * 128: t * 1024 + (m + 1) * 128].T
    return _bf(wp)


def pack_hw2(hW2):
    """hW2 (112,512) -> [128, 4j, 112]: [ph*64+p, j, o] = hW2[o, (2j+ph)*64+p]."""
    hW2 = np.asarray(hW2, np.float32)
    wp = np.empty((128, 4, 112), dtype=np.float32)
    for ph in range(2):
        for j in range(4):
            wp[ph * 64:(ph + 1) * 64, j, :] = hW2[:, (2 * j + ph) * 64:(2 * j + ph + 1) * 64].T
    return _bf(wp)


# ----------------------------------------------------------------- program

def build_program(n_cores):
    nc = bacc.Bacc("TRN2", target_bir_lowering=False, debug=False,
                   num_devices=n_cores)
    core_ids = list(range(n_cores))
    use_cc = n_cores > 1

    ein = lambda name, shape, dt=BF: nc.dram_tensor(name, shape, dt, kind="ExternalInput")
    t1_d = ein("t1", [K1, FPC, 28, 56])
    w1_d = ein("w1p", [K1, 128])
    w2A_d = ein("w2A", [128, 3, 128])
    w2B_d = ein("w2B", [64, 3, 128])
    w3_d = ein("w3p", [128, 9, 2, 128])
    w4_d = ein("w4p", [128, 9, 2, 4, 128])
    wi_d = ein("wip", [128, 4, 8, 128])
    ws_d = ein("wsp", [64, 8, 128])
    wr_d = ein("wrp", [128, 8, 8, 128])
    h1_d = ein("h1p", [128, T, 8, 64])
    h2_d = ein("h2p", [128, 4, 112])
    st_d = ein("stT", [14, T])
    swt_d = ein("swT", [14, 64])
    gb1_d = ein("gb1", [64, 2], F32)
    gb2_d = ein("gb2", [128, 2], F32)
    gb3_d = ein("gb3", [128, 4], F32)    # cols: g_h0, g_h1, b_h0, b_h1
    gb4_d = ein("gb4", [128, 8], F32)    # cols: g x4m then b x4m
    sb_d = ein("sbv", [64, 1], F32)
    bres_d = ein("bres", [128, 8], F32)
    hb1_d = ein("hb1c", [64, 1], F32)    # this core's 64-row slice of hb1
    hb2_d = ein("hb2p", [112, 1], F32)
    mask_d = ein("maskr", [64, 8], F32)   # ones in own rank's column
    maskb_d = ein("maskb", [64, 64], F32)  # [p, r*8+b]: 1 iff b == own rank

    out_d = nc.dram_tensor("out", [112, 1], F32, kind="ExternalOutput")
    if DEBUG_DUMPS:
        dbg_xin = nc.dram_tensor("dbg_xin", [128, T, 8], F32, kind="ExternalOutput")
        dbg_s = nc.dram_tensor("dbg_s", [128, T, 8], F32, kind="ExternalOutput")
        dbg_h = nc.dram_tensor("dbg_h", [64, 8], F32, kind="ExternalOutput")
        dbg_a2 = nc.dram_tensor("dbg_a2", [128, 30, 30], BF, kind="ExternalOutput")
        dbg_a3 = nc.dram_tensor("dbg_a3", [128, 16, 16], BF, kind="ExternalOutput")
        dbg_y4 = nc.dram_tensor("dbg_y4", [128, 4, 16], BF, kind="ExternalOutput")

    if use_cc:
        cc_shapes = [[64, 2], [128, 2], [128, 4], [128, 8], [64, 64], [64, 1]]
        cc_i = [nc.dram_tensor(f"cc{i}i", s, F32) for i, s in enumerate(cc_shapes)]
        cc_o = [nc.dram_tensor(f"cc{i}o", s, F32, addr_space="Shared")
                for i, s in enumerate(cc_shapes)]
        rg_i = nc.dram_tensor("rgi", [128, T, 8], F32)
        rg_o = nc.dram_tensor("rgo", [n_cores, 128, T, 8], F32,
                              addr_space="Shared")

    BTOT = n_cores
    CNT = [float(BTOT * NI * T * hw) for hw in (56 * 56, 14 * 14, 7 * 7, 4 * 4)]

    with tile.TileContext(nc) as tc:
        with (
            tc.tile_pool(name="wts", bufs=1) as wts,
            tc.tile_pool(name="acts", bufs=1) as acts,
            tc.tile_pool(name="tchunk", bufs=2) as tchunk,
            tc.tile_pool(name="ps", bufs=7, space="PSUM") as ps,
            tc.tile_pool(name="ev", bufs=2) as ev,
            tc.tile_pool(name="sq", bufs=1) as sqp,
            tc.tile_pool(name="misc", bufs=1) as misc,
        ):
            def load(dram, shape, dtype=BF):
                tl = wts.tile(shape, dtype, tag=dram.name + "_sb")
                nc.sync.dma_start(tl[:], dram[:])
                return tl

            # ---- collective warmup: tiny AllReduce right at kernel start so
            # the CC cores are hot by the time BN1 stats are ready.
            if use_cc:
                warm_sb = misc.tile([64, 1], F32, tag="warm_sb")
                nc.vector.memset(warm_sb[:], 0.0)
                nc.sync.dma_start(cc_i[5][:], warm_sb[:])
                nc.gpsimd.collective_compute(
                    "AllReduce", ALU.add, ins=[cc_i[5][:]], outs=[cc_o[5][:]],
                    replica_groups=[core_ids])
                warm_g = misc.tile([64, 1], F32, tag="warm_g")
                nc.sync.dma_start(warm_g[:], cc_o[5][:])

            w1a = wts.tile([K1A, 128], BF, tag="w1a")
            nc.sync.dma_start(w1a[:], w1_d[0:K1A, :])
            w1b = wts.tile([K1B, 128], BF, tag="w1b")
            nc.sync.dma_start(w1b[:], w1_d[K1A:K1, :])
            w2A = load(w2A_d, [128, 3, 128])
            w2B = load(w2B_d, [64, 3, 128])
            w3s = load(w3_d, [128, 9, 2, 128])
            w4s = load(w4_d, [128, 9, 2, 4, 128])
            wis = load(wi_d, [128, 4, 8, 128])
            wss = load(ws_d, [64, 8, 128])
            wrs = load(wr_d, [128, 8, 8, 128])
            h1s = load(h1_d, [128, T, 8, 64])
            h2s = load(h2_d, [128, 4, 112])
            sts = load(st_d, [14, T])
            swts = load(swt_d, [14, 64])
            gb1 = load(gb1_d, [64, 2], F32)
            gb2 = load(gb2_d, [128, 2], F32)
            gb3 = load(gb3_d, [128, 4], F32)
            gb4 = load(gb4_d, [128, 8], F32)
            sbv = load(sb_d, [64, 1], F32)
            bres = load(bres_d, [128, 8], F32)
            hb1c = load(hb1_d, [64, 1], F32)
            hb2 = load(hb2_d, [112, 1], F32)
            maskr = load(mask_d, [64, 8], F32)
            maskb = load(maskb_d, [64, 64], F32)

            A2 = acts.tile([128, FPC, 30, 30], BF)
            A3 = acts.tile([128, FPC, 16, 16], BF)
            A4 = acts.tile([128, 2, FPC, 9, 9], BF)
            Y4 = acts.tile([128, 4, FPC, 16], BF)
            # zero only the pad positions each conv reads but never writes
            # (flat per-frame views keep every memset at <=3 total dims)
            A2f = A2.rearrange("p f y x -> p f (y x)")
            A3f = A3.rearrange("p f y x -> p f (y x)")
            A4f = A4.rearrange("p h f y x -> p (h f) (y x)")
            nc.vector.memset(A2f[0:64, :, 0:30], 0.0)        # row 0
            nc.vector.memset(A2f[0:64, :, 30:871:30], 0.0)   # col 0, rows 1..28
            nc.vector.memset(A2f[0:64, :, 59:900:30], 0.0)   # col 29 (dup-shift src)
            nc.vector.memset(A3f[:, :, 0:15], 0.0)           # row 0
            nc.vector.memset(A3f[:, :, 16:240:16], 0.0)      # col 0, rows 1..14
            nc.vector.memset(A4f[:, :, 0:9], 0.0)            # row 0
            nc.vector.memset(A4f[:, :, 72:81], 0.0)          # row 8
            nc.vector.memset(A4f[:, :, 9:72:9], 0.0)         # col 0, rows 1..7
            nc.vector.memset(A4f[:, :, 17:72:9], 0.0)        # col 8, rows 1..7

            s1acc = acts.tile([128, FPC * 4], F32)
            q1acc = acts.tile([128, FPC], F32)
            s2acc = acts.tile([128, 16], F32)
            q2acc = acts.tile([128, 16], F32)
            s3acc = acts.tile([128, 8], F32)
            q3acc = acts.tile([128, 8], F32)
            s4acc = acts.tile([128, 4], F32)
            q4acc = acts.tile([128, 4], F32)

            # ================ conv1 + maxpool, per frame
            r_groups = [(0, 8), (8, 16), (16, 24), (24, 28)]
            for f in range(FPC):
                ta = tchunk.tile([K1A, 28, 56], BF, tag="ta")
                nc.sync.dma_start(ta[:], t1_d[0:K1A, f, :, :])
                tb = tchunk.tile([K1B, 28, 56], BF, tag="tb")
                nc.sync.dma_start(tb[:], t1_d[K1A:K1, f, :, :])
                yf = ev.tile([128, 28, 56], BF, tag="yf")
                for gi, (r0, r1) in enumerate(r_groups):
                    pt = ps.tile([128, r1 - r0, 56], F32, tag="ps")
                    nc.tensor.matmul(pt[:], w1a[:], ta[:, r0:r1, :],
                                     start=True, stop=False)
                    nc.tensor.matmul(pt[:], w1b[:], tb[:, r0:r1, :],
                                     start=False, stop=True)
                    nc.scalar.activation(yf[:, r0:r1, :], pt[:], AF.Copy,
                                         accum_out=s1acc[:, f * 4 + gi: f * 4 + gi + 1])
                # sum of squares on DVE (4x mode) instead of a second ACT pass
                sq1 = sqp.tile([128, 28, 56], BF, tag="sq1")
                nc.vector.scalar_tensor_tensor(
                    out=sq1[:], in0=yf[:], scalar=1.0, in1=yf[:],
                    op0=ALU.mult, op1=ALU.mult,
                    accum_out=q1acc[:, f:f + 1])
                # maxpool 3x3 s2 p1; rows y=2r+ry live as (partition ry, free r)
                ax = ev.tile([128, 28, 28], BF, tag="ax")
                nc.vector.tensor_tensor(out=ax[:], in0=yf[:, :, 0:56:2],
                                        in1=yf[:, :, 1:56:2], op=ALU.max)
                px = ev.tile([128, 28, 28], BF, tag="px")
                nc.vector.tensor_copy(px[:, :, 0:1], ax[:, :, 0:1])
                nc.vector.tensor_tensor(out=px[:, :, 1:28], in0=ax[:, :, 1:28],
                                        in1=yf[:, :, 1:54:2], op=ALU.max)
                # copy upper-half rows to base-0 (walrus: TT inputs must
                # share base partition)
                pxb = ev.tile([64, 28, 28], BF, tag="pxb")
                nc.vector.tensor_copy(pxb[:], px[64:128, :, :])
                cy = ev.tile([64, 28, 28], BF, tag="cy")
                nc.vector.tensor_tensor(out=cy[:], in0=px[0:64, :, :],
                                        in1=pxb[:], op=ALU.max)
                nc.vector.tensor_copy(A2[0:64, f, 1:2, 1:29], cy[:, 0:1, :])
                nc.vector.tensor_tensor(out=A2[0:64, f, 2:29, 1:29],
                                        in0=cy[:, 1:28, :],
                                        in1=pxb[:, 0:27, :], op=ALU.max)

            # ---------------- BN helpers
            def bn_finalize(s_red, q_red, gb, cnt, P, K, lname):
                m = misc.tile([P, K], F32, tag=lname + "_m")
                nc.vector.tensor_scalar_mul(m[:], s_red[:], 1.0 / cnt)
                msq = misc.tile([P, K], F32, tag=lname + "_msq")
                nc.vector.tensor_mul(msq[:], m[:], m[:])
                v = misc.tile([P, K], F32, tag=lname + "_v")
                nc.vector.scalar_tensor_tensor(out=v[:], in0=q_red[:], scalar=1.0 / cnt,
                                               in1=msq[:], op0=ALU.mult,
                                               op1=ALU.subtract)
                ve = misc.tile([P, K], F32, tag=lname + "_ve")
                nc.vector.tensor_scalar_add(ve[:], v[:], float(BN_EPS))
                rc = misc.tile([P, K], F32, tag=lname + "_rc")
                nc.vector.reciprocal(rc[:], ve[:])
                rinv = misc.tile([P, K], F32, tag=lname + "_rinv")
                nc.scalar.activation(rinv[:], rc[:], AF.Sqrt)
                a = misc.tile([P, K], F32, tag=lname + "_a")
                nc.vector.tensor_mul(a[:], rinv[:], gb[:, 0:K])
                tmv = misc.tile([P, K], F32, tag=lname + "_tmv")
                nc.vector.tensor_mul(tmv[:], m[:], a[:])
                b = misc.tile([P, K], F32, tag=lname + "_b")
                nc.vector.tensor_sub(b[:], gb[:, K:2 * K], tmv[:])
                return a, b

            def allreduce(idx, src, P, K, lname):
                if not use_cc:
                    return src
                nc.sync.dma_start(cc_i[idx][:], src[:])
                nc.gpsimd.collective_compute(
                    "AllReduce", ALU.add, ins=[cc_i[idx][:]], outs=[cc_o[idx][:]],
                    replica_groups=[core_ids])
                g = misc.tile([P, K], F32, tag=lname + "_g")
                nc.sync.dma_start(g[:], cc_o[idx][:])
                return g

            # ---------------- BN1 -> A2
            s1r = misc.tile([128, 1], F32, tag="s1r")
            nc.vector.reduce_sum(s1r[:], s1acc[:], axis=AX.X)
            q1r = misc.tile([128, 1], F32, tag="q1r")
            nc.vector.reduce_sum(q1r[:], q1acc[:], axis=AX.X)
            st1 = misc.tile([64, 2], F32, tag="st1")
            s1hi = misc.tile([64, 2], F32, tag="s1hi")
            nc.vector.tensor_copy(s1hi[:, 0:1], s1r[64:128, :])
            nc.vector.tensor_copy(s1hi[:, 1:2], q1r[64:128, :])
            nc.vector.tensor_add(st1[:, 0:1], s1r[0:64, :], s1hi[:, 0:1])
            nc.vector.tensor_add(st1[:, 1:2], q1r[0:64, :], s1hi[:, 1:2])
            if use_cc:
                # fold 0*warm_g into the stats so the warmup collective is
                # not dead code (also serializes BN1 after the warmup).
                nc.vector.scalar_tensor_tensor(
                    out=st1[:, 0:1], in0=warm_g[:], scalar=0.0,
                    in1=st1[:, 0:1], op0=ALU.mult, op1=ALU.add)
            g1 = allreduce(0, st1, 64, 2, "l1")
            a1, b1 = bn_finalize(g1[:, 0:1], g1[:, 1:2], gb1, CNT[0], 64, 1, "l1")
            # BN+relu over the whole pooled interior in one op, then one big
            # x-shifted duplicate for conv2's kx pairing.
            nc.scalar.activation(A2[0:64, :, 1:29, 1:29], A2[0:64, :, 1:29, 1:29],
                                 AF.Relu, bias=b1[:, 0:1], scale=a1[:, 0:1])
            # x-shifted duplicate as one flat packed copy (col 29 of the src
            # half is zeroed above so the row wrap is harmless; the only
            # wrapped positions land in cells conv2 never reads)
            nc.vector.tensor_copy(A2f[64:128, :, 0:870], A2f[0:64, :, 1:871])

            # ================ conv2
            for g in range(FPC // 2):
                f0 = g * 2
                pt = ps.tile([128, 2, 14, 14], F32, tag="ps")
                for ky in range(3):
                    nc.tensor.matmul(pt[:], w2A[:, ky, :],
                                     A2[:, f0:f0 + 2, ky:ky + 28:2, 0:28:2],
                                     start=(ky == 0), stop=False)
                    nc.tensor.matmul(pt[:], w2B[:, ky, :],
                                     A2[0:64, f0:f0 + 2, ky:ky + 28:2, 2:30:2],
                                     start=False, stop=(ky == 2))
                nc.scalar.activation(A3[:, f0:f0 + 2, 1:15, 1:15], pt[:],
                                     AF.Copy, accum_out=s2acc[:, g:g + 1])
                sq2 = sqp.tile([128, 2, 14, 14], BF, tag="sq2")
                nc.scalar.activation(sq2[:], A3[:, f0:f0 + 2, 1:15, 1:15],
                                     AF.Square, accum_out=q2acc[:, g:g + 1])

            st2 = misc.tile([128, 2], F32, tag="st2")
            nc.vector.reduce_sum(st2[:, 0:1], s2acc[:], axis=AX.X)
            nc.vector.reduce_sum(st2[:, 1:2], q2acc[:], axis=AX.X)
            g2 = allreduce(1, st2, 128, 2, "l2")
            a2, b2 = bn_finalize(g2[:, 0:1], g2[:, 1:2], gb2, CNT[1], 128, 1, "l2")
            nc.scalar.activation(A3[:, :, 1:15, 1:15], A3[:, :, 1:15, 1:15],
                                 AF.Relu, bias=b2[:, 0:1], scale=a2[:, 0:1])

            # ================ conv3
            for m in range(2):
                for g in range(4):
                    f0 = g * 8
                    pt = ps.tile([128, 8, 7, 7], F32, tag="ps")
                    for tap in range(9):
                        ky, kx = tap // 3, tap % 3
                        nc.tensor.matmul(pt[:], w3s[:, tap, m, :],
                                         A3[:, f0:f0 + 8, ky:ky + 14:2, kx:kx + 14:2],
                                         start=(tap == 0), stop=(tap == 8))
                    ci = m * 4 + g
                    nc.scalar.activation(A4[:, m, f0:f0 + 8, 1:8, 1:8], pt[:],
                                         AF.Copy, accum_out=s3acc[:, ci:ci + 1])
                    sq3 = sqp.tile([128, 8, 7, 7], BF, tag="sq3")
                    nc.scalar.activation(sq3[:], A4[:, m, f0:f0 + 8, 1:8, 1:8],
                                         AF.Square, accum_out=q3acc[:, ci:ci + 1])

            st3 = misc.tile([128, 4], F32, tag="st3")
            for m in range(2):
                nc.vector.reduce_sum(st3[:, m:m + 1], s3acc[:, m * 4:(m + 1) * 4],
                                     axis=AX.X)
                nc.vector.reduce_sum(st3[:, 2 + m:3 + m], q3acc[:, m * 4:(m + 1) * 4],
                                     axis=AX.X)
            g3 = allreduce(2, st3, 128, 4, "l3")
            a3, b3 = bn_finalize(g3[:, 0:2], g3[:, 2:4], gb3, CNT[2], 128, 2, "l3")
            for h in range(2):
                nc.scalar.activation(
                    A4[:, h, :, 1:8, 1:8], A4[:, h, :, 1:8, 1:8],
                    AF.Relu, bias=b3[:, h:h + 1], scale=a3[:, h:h + 1])

            # ================ conv4
            for m in range(4):
                pt = ps.tile([128, FPC, 4, 4], F32, tag="ps")
                first = True
                for tap in range(9):
                    ky, kx = tap // 3, tap % 3
                    for h in range(2):
                        nc.tensor.matmul(pt[:], w4s[:, tap, h, m, :],
                                         A4[:, h, :, ky:ky + 7:2, kx:kx + 7:2],
                                         start=first, stop=(tap == 8 and h == 1))
                        first = False
                nc.scalar.activation(
                    Y4[:, m, :, :].rearrange("p f (y x) -> p f y x", y=4),
                    pt[:], AF.Copy, accum_out=s4acc[:, m:m + 1])
                sq4 = sqp.tile([128, FPC, 16], BF, tag="sq4")
                nc.vector.scalar_tensor_tensor(
                    out=sq4[:], in0=Y4[:, m, :, :], scalar=1.0, in1=Y4[:, m, :, :],
                    op0=ALU.mult, op1=ALU.mult,
                    accum_out=q4acc[:, m:m + 1])

            st4 = misc.tile([128, 8], F32, tag="st4")
            nc.vector.tensor_copy(st4[:, 0:4], s4acc[:])
            nc.vector.tensor_copy(st4[:, 4:8], q4acc[:])
            g4 = allreduce(3, st4, 128, 8, "l4")
            a4, b4 = bn_finalize(g4[:, 0:4], g4[:, 4:8], gb4, CNT[3], 128, 4, "l4")

            # BN4+relu -> avgpool(16px) -> image-mean -> feats [128,(4m,16t)]
            S4 = acts.tile([128, 4, FPC], F32)
            for m in range(4):
                r4m = ev.tile([128, FPC, 16], F32, tag="r4m")
                nc.scalar.activation(r4m[:], Y4[:, m, :, :], AF.Relu,
                                     bias=b4[:, m:m + 1], scale=a4[:, m:m + 1])
                nc.vector.reduce_sum(S4[:, m, :], r4m[:], axis=AX.X)
            fsum = acts.tile([128, 4, T], F32)
            nc.vector.tensor_add(fsum[:], S4[:, :, 0:T], S4[:, :, T:FPC])
            FIb = acts.tile([128, 4, T], BF)
            nc.vector.tensor_scalar_mul(FIb[:], fsum[:], 1.0 / 32.0)

            # ================ state embedding (own b): SE [64, 16] bf16
            pse = ps.tile([64, T], F32, tag="ps")
            nc.tensor.matmul(pse[:], swts[:], sts[:], start=True, stop=True)
            SE = acts.tile([64, T], BF)
            nc.scalar.activation(SE[:], pse[:], AF.Relu, bias=sbv[:, 0:1])

            # ================ xin[p, t, m] = (Win @ step_inputs)_t + b_res
            xin = acts.tile([128, T, 8], F32)
            for m in range(8):
                px2 = ps.tile([128, T], F32, tag="ps")
                for ki in range(4):
                    nc.tensor.matmul(px2[:], wis[:, ki, m, :], FIb[:, ki, :],
                                     start=(ki == 0), stop=False)
                nc.tensor.matmul(px2[:], wss[:, m, :], SE[:], start=False, stop=True)
                nc.scalar.activation(xin[:, :, m], px2[:], AF.Identity,
                                     bias=bres[:, m:m + 1])

            # ================ reservoir scan (own b), s = 2r form:
            # s_t = 0.5*s_{t-1} + tanh(xin_t + (0.5*Wres) s_{t-1})
            s_hist = acts.tile([128, T, 8], F32)
            sb16 = acts.tile([128, T, 8], BF)
            szero = acts.tile([128, 8], F32)
            szero16 = acts.tile([128, 8], BF)
            nc.vector.memset(szero[:], 0.0)
            nc.vector.memset(szero16[:], 0.0)
            for t in range(T):
                pr = ps.tile([128, 8], F32, tag="ps")
                for m in range(8):
                    for k in range(8):
                        rhs = (szero16[:, k:k + 1] if t == 0
                               else sb16[:, t - 1, k:k + 1])
                        nc.tensor.matmul(pr[:, m:m + 1], wrs[:, k, m, :],
                                         rhs, start=(k == 0), stop=(k == 7))
                u = ev.tile([128, 8], F32, tag="scan_u")
                nc.vector.scalar_tensor_tensor(
                    out=u[:], in0=pr[:], scalar=1.0, in1=xin[:, t, :],
                    op0=ALU.mult, op1=ALU.add)
                th = ev.tile([128, 8], F32, tag="scan_th")
                nc.scalar.activation(th[:], u[:], AF.Tanh)
                sprev = szero[:] if t == 0 else s_hist[:, t - 1, :]
                nc.vector.scalar_tensor_tensor(
                    out=s_hist[:, t, :], in0=sprev, scalar=0.5, in1=th[:],
                    op0=ALU.mult, op1=ALU.add)
                nc.vector.tensor_copy(sb16[:, t, :], s_hist[:, t, :])

            # ================ head. hW1 is sharded by output rows; every core
            # computes its 64-row slice for ALL batches, so first AllGather the
            # (tiny) reservoir histories, then redistribute full-H per batch
            # with a masked AllReduce.
            nc.sync.dma_start(rg_i[:], s_hist[:])
            nc.gpsimd.collective_compute(
                "AllGather", ALU.bypass, ins=[rg_i[:]], outs=[rg_o[:]],
                replica_groups=[core_ids])
            Rf = acts.tile([128, n_cores, T, 8], F32)
            nc.sync.dma_start(Rf[:], rg_o[:].rearrange("b p t m -> p b t m"))
            Rb16 = acts.tile([128, n_cores, T, 8], BF)
            nc.vector.tensor_copy(Rb16[:], Rf[:])
            ph = ps.tile([64, 8], F32, tag="ps")
            first = True
            for t in range(T):
                for m in range(8):
                    nc.tensor.matmul(ph[:], h1s[:, t, m, :], Rb16[:, :, t, m],
                                     start=first, stop=(t == T - 1 and m == 7))
                    first = False
            hcs = misc.tile([64, 8], F32, tag="hcs")
            nc.scalar.activation(hcs[:], ph[:], AF.Identity, bias=hb1c[:, 0:1])
            # buf[p, r*8+b] = hcs[p, b] iff r == own rank, else 0
            hbuf = misc.tile([64, 64], F32, tag="hbuf")
            for r in range(n_cores):
                nc.vector.scalar_tensor_tensor(
                    out=hbuf[:, r * 8:(r + 1) * 8], in0=hcs[:],
                    scalar=maskr[:, r:r + 1],
                    in1=szero[0:64, 0:8], op0=ALU.mult, op1=ALU.add)
            gH = allreduce(4, hbuf, 64, 64, "lH")
            # select own batch's column: msel[p, r*8+b] = gH * (b == rank)
            msel = misc.tile([64, 64], F32, tag="msel")
            nc.vector.tensor_mul(msel[:], gH[:], maskb[:])
            Hsel = misc.tile([64, 8], F32, tag="Hsel")
            nc.vector.reduce_sum(
                Hsel[:], msel[:].rearrange("p (r b) -> p r b", b=8), axis=AX.X)
            H64 = misc.tile([64, 8], BF, tag="H64")
            nc.scalar.activation(H64[:], Hsel[:], AF.Relu)
            H128 = misc.tile([128, 4], BF, tag="H128")
            nc.vector.tensor_copy(H128[0:64, :], H64[:, 0:8:2])
            nc.vector.tensor_copy(H128[64:128, :], H64[:, 1:8:2])

            po = ps.tile([112, 1], F32, tag="ps")
            for j in range(4):
                nc.tensor.matmul(po[:], h2s[:, j, :], H128[:, j:j + 1],
                                 start=(j == 0), stop=(j == 3))
            OutS = acts.tile([112, 1], F32)
            nc.scalar.activation(OutS[:], po[:], AF.Identity, bias=hb2[:, 0:1])
            nc.sync.dma_start(out_d[:], OutS[:])
            if DEBUG_DUMPS:
                nc.sync.dma_start(dbg_xin[:], xin[:])
                nc.sync.dma_start(dbg_s[:], s_hist[:])
                nc.sync.dma_start(dbg_h[:], Hsel[:])
                nc.sync.dma_start(dbg_a2[:], A2[:, 0, :, :])
                nc.sync.dma_start(dbg_a3[:], A3[:, 0, :, :])
                nc.sync.dma_start(dbg_y4[:], Y4[:, :, 0, :].rearrange("p m x -> p m x"))

    nc.compile()
    return nc


# ----------------------------------------------------------------- host driver

_CACHE = {}


def make_in_map(inputs, core):
    b = core
    imgs = np.asarray(inputs["images_seq"], np.float32)[b].reshape(FPC, 3, 112, 112)
    wA, wB = pack_w2(inputs["w2"])
    wi, ws = pack_win(inputs["Win"])
    f32 = lambda x: np.asarray(x, np.float32)
    mask = np.zeros((64, 8), dtype=np.float32)
    mask[:, core] = 1.0
    maskb = np.zeros((64, 8, 8), dtype=np.float32)
    maskb[:, :, core] = 1.0
    d = {
        "t1": conv1_im2col(imgs),
        "w1p": pack_w1(inputs["w1"]),
        "w2A": wA, "w2B": wB,
        "w3p": pack_w3(inputs["w3"]),
        "w4p": pack_w4(inputs["w4"]),
        "wip": wi, "wsp": ws,
        "wrp": pack_wres(inputs["Wres"]),
        "h1p": pack_hw1_slice(inputs["hW1"], core),
        "h2p": pack_hw2(inputs["hW2"]),
        "stT": _bf(f32(inputs["state_seq"])[b].T),
        "swT": _bf(f32(inputs["sW"]).T),
        "gb1": np.stack([f32(inputs["g1"]), f32(inputs["be1"])], axis=1),
        "gb2": np.stack([f32(inputs["g2"]), f32(inputs["be2"])], axis=1),
        "gb3": np.concatenate([f32(inputs["g3"]).reshape(2, 128).T,
                               f32(inputs["be3"]).reshape(2, 128).T], axis=1),
        "gb4": np.concatenate([f32(inputs["g4"]).reshape(4, 128).T,
                               f32(inputs["be4"]).reshape(4, 128).T], axis=1),
        "sbv": f32(inputs["sb"]).reshape(64, 1),
        "bres": f32(inputs["b_res"]).reshape(8, 128).T.copy(),
        "hb1c": f32(inputs["hb1"])[64 * core:64 * (core + 1)].reshape(64, 1),
        "hb2p": f32(inputs["hb2"]).reshape(112, 1),
        "maskr": mask,
        "maskb": maskb.reshape(64, 64),
    }
    return d


def run(inputs, n_cores=8, **kw):
    core_ids = list(range(n_cores))
    if n_cores not in _CACHE:
        _CACHE[n_cores] = build_program(n_cores)
    nc = _CACHE[n_cores]
    in_maps = [make_in_map(inputs, c) for c in core_ids]
    res = run_bass_kernel_spmd(nc, in_maps, core_ids, **kw)
    rows = [np.asarray(res.results[c]["out"], np.float32).reshape(112)
            for c in core_ids]
    return np.stack(rows, axis=0), res


def kernel(**inputs):
    out, _ = run(inputs, n_cores=8)
    return out.reshape(8, 8, 14)
